# revision 24
# baseline (speedup 1.0000x reference)
"""Trainium2 Bass kernel for CustomLSTMForecast, v5.

B=512, T=256, I=256, H=512. Data-parallel: batch sharded 8 ways (64
rows/core), LSTM + fc weights replicated.

Hardware-profiling showed per-step cost on this runtime is dominated by
cross-engine dependency edges (~0.5-0.9us each), not engine cycles, so
v5 minimizes instructions and sync edges per step:

  - gates as two full-width PSUM tiles: pA = [i-hat rows 0:64; f-hat
    rows 64:128], pB = [chat; o-hat], each [128, 512] (one bank), fed by
    K=2 bias matmul (start=True for the whole bank) + 8 x matmuls + 16
    h matmuls, all N=512 with M=64 column-pair packing.
  - ACT: one sigmoid over pA, tanh over pB[0:64], sigmoid over
    pB[64:128], tanh(c), and the hT copy.
  - DVE: u1 = f*c (base 64), u2 = i*chat -> PSUM f32 (the only
    partition-base crossing, exempt from the same-start rule), c = u1 +
    u2, h = o*tanh(c). All bf16 elementwise.
  - x for all steps is SBUF-resident via one bulk DMA; PSUM gate tiles
    are seeded two steps ahead so x/bias matmuls fill PE idle.
"""
from contextlib import ExitStack

import numpy as np

import concourse.bass as bass
import concourse.tile as tile
from concourse import bacc, mybir
from concourse.bass_utils import run_bass_kernel_spmd

F32 = mybir.dt.float32
BF16 = mybir.dt.bfloat16
AF = mybir.ActivationFunctionType
ALU = mybir.AluOpType

B, T, I, H = 512, 256, 256, 512
EDT = BF16                # elementwise/ladder dtype
NCORES = 8
BC = B // NCORES          # 64 batch rows per core
KH = H // 128             # 4 hidden k-chunks
KX = I // 128             # 2 input k-chunks
NK = KH + KX              # 6 weight k-chunks (bias handled separately)

# (tile, row) -> gate block in W_w row order f=0, i=1, o=2, chat=3
TILE_GATES = ((1, 0), (3, 2))   # tile A: (i, f); tile B: (chat, o)

_CACHE = {}


def _build(nsteps=T):
    if nsteps in _CACHE:
        return _CACHE[nsteps]
    nc = bacc.Bacc("TRN2", target_bir_lowering=False, debug=False,
                   num_devices=NCORES)
    d_x = nc.dram_tensor("xT", [128, nsteps, KX, BC], BF16,
                         kind="ExternalInput").ap()
    d_w = nc.dram_tensor("W", [NK, 128, 2, 2, 512], BF16,
                         kind="ExternalInput").ap()
    d_bsel = nc.dram_tensor("bsel", [2, 128], BF16,
                            kind="ExternalInput").ap()
    d_bias = nc.dram_tensor("bias2", [2, 2, 512], BF16,
                            kind="ExternalInput").ap()
    d_eye = nc.dram_tensor("eye", [BC, BC], EDT, kind="ExternalInput").ap()
    d_fcw = nc.dram_tensor("fcw", [BC, H], F32, kind="ExternalInput").ap()
    d_fcb = nc.dram_tensor("fcb", [BC, 1], F32, kind="ExternalInput").ap()
    d_out = nc.dram_tensor("out", [BC, 1], F32, kind="ExternalOutput").ap()

    with tile.TileContext(nc) as tc, ExitStack() as ctx:
        _body(tc, ctx, nsteps, d_x, d_w, d_bsel, d_bias, d_eye, d_fcw,
              d_fcb, d_out)
    nc.compile()
    _CACHE[nsteps] = nc
    return nc


def _body(tc, ctx, nsteps, d_x, d_w, d_bsel, d_bias, d_eye, d_fcw, d_fcb,
          d_out):
    nc = tc.nc
    const = ctx.enter_context(tc.tile_pool(name="const", bufs=1))
    gact = ctx.enter_context(tc.tile_pool(name="gact", bufs=2))
    state = ctx.enter_context(tc.tile_pool(name="state", bufs=2))
    psAB = ctx.enter_context(tc.tile_pool(name="psAB", bufs=2, space="PSUM"))
    psU = ctx.enter_context(tc.tile_pool(name="psU", bufs=1, space="PSUM"))
    psT = ctx.enter_context(tc.tile_pool(name="psT", bufs=2, space="PSUM"))

    sW = const.tile([128, NK, 2, 2, 512], BF16)
    nc.sync.dma_start(out=sW[:], in_=d_w.rearrange("k p a r n -> p k a r n"))
    sX = const.tile([128, nsteps, KX, BC], BF16)
    nc.sync.dma_start(out=sX[:], in_=d_x)
    s_bsel = const.tile([2, 128], BF16)
    nc.sync.dma_start(out=s_bsel[:], in_=d_bsel)
    s_bias = const.tile([2, 2, 512], BF16)
    nc.sync.dma_start(out=s_bias[:], in_=d_bias)
    s_eye = const.tile([128, BC], EDT)
    nc.sync.dma_start(out=s_eye[64:128, :], in_=d_eye)
    s_fcw = const.tile([128, H], F32)
    nc.sync.dma_start(out=s_fcw[64:128, :], in_=d_fcw)
    s_fcb = const.tile([128, 1], F32)
    nc.sync.dma_start(out=s_fcb[64:128, :], in_=d_fcb)

    c_prev = state.tile([128, H], EDT, tag="c")
    nc.vector.memset(c_prev[64:128, :], 0.0)

    tiles = {}

    def alloc_tiles(t):
        pa = psAB.tile([128, 512], F32, tag="A", name=f"pA{t % 2}")
        pb = psAB.tile([128, 512], F32, tag="B", name=f"pB{t % 2}")
        tiles[t] = (pa, pb)

    def emit_bias(pA, pB):
        for ti, p in ((0, pA), (1, pB)):
            nc.tensor.matmul(p[:, :], s_bsel[:], s_bias[:, ti, :],
                             start=True, stop=False)

    def emit_x(t, pA, pB, stop, kxs=tuple(range(KX))):
        for kx in kxs:
            for ti, p in ((0, pA), (1, pB)):
                for row in range(2):
                    nc.tensor.matmul(p[64 * row:64 * row + 64, :],
                                     sX[:, t, kx, :],
                                     sW[:, KH + kx, ti, row, :],
                                     start=False,
                                     stop=stop and kx == KX - 1)

    def emit_h(pA, pB, hT):
        for ti, p in ((0, pA), (1, pB)):
            for k in range(KH):
                for row in range(2):
                    nc.tensor.matmul(p[64 * row:64 * row + 64, :],
                                     hT[:, 64 * k:64 * k + 64],
                                     sW[:, k, ti, row, :],
                                     start=False, stop=(k == KH - 1))

    # prologue: steps 0 and 1 tiles seeded ahead (two-step pipeline)
    alloc_tiles(0)
    emit_bias(*tiles[0])
    emit_x(0, *tiles[0], stop=True)
    if nsteps > 1:
        alloc_tiles(1)
        emit_bias(*tiles[1])
        emit_x(1, *tiles[1], stop=False)

    hT = None
    h = None
    for t in range(nsteps):
        last = t == nsteps - 1
        pA, pB = tiles.pop(t)
        if t > 0:
            emit_h(pA, pB, hT)

        # sA = [i-hat; f-hat]; tcb = tanh(chat) at base 0; sO holds o-hat
        # at base 64 (rows 64:128 of its tile)
        sA = gact.tile([128, 512], EDT, tag="sA")
        nc.scalar.activation(sA[:, :], pA[:, :], AF.Sigmoid)
        tcb = gact.tile([BC, H], EDT, tag="tcb")
        nc.scalar.activation(tcb[:, :], pB[0:64, :], AF.Tanh)
        sO = gact.tile([128, H], EDT, tag="sO")
        nc.scalar.activation(sO[64:128, :], pB[64:128, :], AF.Sigmoid)

        u1 = gact.tile([128, H], EDT, tag="u1")
        nc.vector.tensor_mul(u1[64:128, :], sA[64:128, :], c_prev[64:128, :])
        u2p = psU.tile([64, 512], F32, tag="u2")
        nc.vector.tensor_mul(u2p[:, :], sA[0:64, :], tcb[:, :])
        c_new = state.tile([128, H], EDT, tag="c")
        nc.vector.tensor_add(c_new[64:128, :], u1[64:128, :], u2p[:, :])
        tch = gact.tile([128, H], EDT, tag="tch")
        nc.scalar.activation(tch[64:128, :], c_new[64:128, :], AF.Tanh)
        h = gact.tile([128, H], EDT, tag="h")
        nc.vector.tensor_mul(h[64:128, :], sO[64:128, :], tch[64:128, :])
        c_prev = c_new

        if not last:
            t2 = t + 2
            if t2 < nsteps:
                alloc_tiles(t2)
                emit_bias(*tiles[t2])
                emit_x(t2, *tiles[t2], stop=False, kxs=(0,))
            pT = psT.tile([128, 256], EDT, tag="T")
            hTn = state.tile([128, 256], BF16, tag="hT")
            for j in range(KH):
                nc.tensor.transpose(
                    pT[:, 64 * j:64 * j + 64],
                    h[64:128, 128 * j:128 * j + 128],
                    s_eye[64:128, :])
            nc.scalar.copy(hTn[:, :], pT[:, :])
            if t2 < nsteps:
                emit_x(t2, *tiles[t2], stop=False, kxs=(1,))
            hT = hTn

    # fc head: out = h @ fc_w.T + fc_b (all at partition base 64)
    m = gact.tile([128, H], F32, tag="fcm")
    nc.vector.tensor_mul(m[64:128, :], h[64:128, :], s_fcw[64:128, :])
    r = gact.tile([128, 1], F32, tag="fcr")
    nc.vector.tensor_reduce(r[64:128, :], m[64:128, :],
                            axis=mybir.AxisListType.X, op=ALU.add)
    ro = gact.tile([128, 1], F32, tag="fco")
    nc.vector.tensor_add(ro[64:128, :], r[64:128, :], s_fcb[64:128, :])
    nc.sync.dma_start(out=d_out, in_=ro[64:128, :])


def _prep_core_inputs(x, W_w, W_b, fc_w, fc_b, core, nsteps=T):
    """Host-side shard + relayout for one core."""
    xs = x[core * BC:(core + 1) * BC, :nsteps]          # [BC, t, I]
    xt = np.ascontiguousarray(xs.transpose(1, 2, 0))    # [t, I, BC]
    xt = xt.reshape(nsteps, KX, 128, BC).transpose(2, 0, 1, 3)
    xt = np.ascontiguousarray(xt)                       # [128, t, KX, BC]

    wfull = W_w.T                                       # [768, 2048]
    wt = np.zeros((NK, 128, 2, 2, 512), dtype=np.float32)
    bias2 = np.zeros((2, 2, 512), dtype=np.float32)
    for ti in range(2):
        for row in range(2):
            g = TILE_GATES[ti][row]
            for k in range(NK):
                wt[k, :, ti, row, :] = (
                    wfull[128 * k:128 * (k + 1), 512 * g:512 * (g + 1)])
            bias2[row, ti, :] = W_b[512 * g:512 * (g + 1)]

    bsel = np.zeros((2, 128), dtype=np.float32)
    bsel[0, 0:64] = 1.0
    bsel[1, 64:128] = 1.0
    eye = np.eye(BC, dtype=np.float32)
    fcw = np.ascontiguousarray(np.broadcast_to(fc_w.reshape(1, H), (BC, H)))
    fcb = np.full((BC, 1), np.float32(fc_b[0]), dtype=np.float32)

    import ml_dtypes
    bf = ml_dtypes.bfloat16
    edt = np.float32 if EDT == F32 else bf
    return {"xT": xt.astype(bf), "W": wt.astype(bf),
            "bsel": bsel.astype(bf), "bias2": bias2.astype(bf),
            "eye": eye.astype(edt), "fcw": fcw, "fcb": fcb}


def kernel(x, W_w, W_b, fc_w, fc_b):
    x = np.asarray(x, dtype=np.float32)
    W_w = np.asarray(W_w, dtype=np.float32)
    W_b = np.asarray(W_b, dtype=np.float32)
    fc_w = np.asarray(fc_w, dtype=np.float32)
    fc_b = np.asarray(fc_b, dtype=np.float32)

    nc = _build(T)
    in_maps = [_prep_core_inputs(x, W_w, W_b, fc_w, fc_b, c)
               for c in range(NCORES)]
    res = run_bass_kernel_spmd(nc, in_maps, list(range(NCORES))).results
    return np.concatenate([res[c]["out"] for c in range(NCORES)], axis=0)
